# revision 25
# baseline (speedup 1.0000x reference)
"""DISCO downsample conv (3x3, stride 2, pad 1) on 8 Trainium2 NeuronCores.

Strategy:
  - Effective weights w[o,i,kh,kw] = sum_b coeff[o,i,b]*basis[b,kh,kw] are tiny:
    computed on host, shipped per-tap transposed as wt[i, tap, o] (fp16).
  - x is zero-padded (H+2, W+2) on host, W phase-split into [even | odd]
    columns so every conv tap reads a contiguous run of 256 columns, and cast
    to fp16 (halves HBM traffic vs fp32 at the same 1 cycle/row TensorE rate
    as bf16, with 8x finer mantissa; PSUM accumulation stays fp32).
  - Sharding: 8 shards = (batch b in 0..3) x (H half in 0..1). Each core gets
    padded rows [256*h, 256*h + 257) of batch b -- the 1-row halo is part of
    the shard, so no inter-core communication is needed.
  - Per core: stream 4 row-blocks (32 output rows each = 65 input rows),
    each loaded in chunks so compute overlaps the load. For each pair of
    output rows: one PSUM bank [96, 512], accumulate 9 matmuls (K=96
    in-channels, M=96 out-channels, N=512 pixels), then ScalarE adds bias
    while copying PSUM -> SBUF (fp16), and rows are flushed to HBM in
    half-blocks on the ACT HWDGE ring.
"""

import os
import sys
import types

import numpy as np


# ----------------------------------------------------------------------------
# Environment bootstrap (self-contained: no reads from /root/problem).
# ----------------------------------------------------------------------------
def _ensure_paths():
    for p in (
        "/root/.axon_site",
        "/root/.axon_site/_ro/trn_rl_repo",
        "/root/.axon_site/_ro/pypackages",
        "/opt/trn_rl_repo",
    ):
        if os.path.isdir(p) and p not in sys.path:
            sys.path.append(p)


_ensure_paths()

import ml_dtypes  # noqa: E402


def _install_ntff_hook():
    """Register the NTFF profile hook (used when tracing; harmless otherwise)."""
    try:
        import antenv
    except ImportError:
        return
    if "antenv.axon_hooks" not in sys.modules:
        hooks_mod = types.ModuleType("antenv.axon_hooks")
        _hook = [None]
        hooks_mod.set_axon_ntff_profile_hook = lambda h: _hook.__setitem__(0, h)
        hooks_mod.get_axon_ntff_profile_hook = lambda: _hook[0]
        sys.modules["antenv.axon_hooks"] = hooks_mod
        antenv.axon_hooks = hooks_mod
    from antenv.axon_hooks import (
        get_axon_ntff_profile_hook,
        set_axon_ntff_profile_hook,
    )

    if get_axon_ntff_profile_hook() is None:
        try:
            from trn_agent_boot.trn_boot import _ntff_profile_via_ctypes

            so = "/opt/axon/libaxon_pjrt.so"
            if os.path.exists(so):
                set_axon_ntff_profile_hook(_ntff_profile_via_ctypes(so))
        except Exception:
            pass


_install_ntff_hook()

import concourse.bass as bass  # noqa: E402
import concourse.tile as tile  # noqa: E402
from concourse import bacc, mybir  # noqa: E402
import concourse.bass_utils as _bu  # noqa: E402

# Artifact upload needs a bucket that isn't reachable here; keep traces local.
_bu.upload_artifacts = lambda tmpdir: f"local:{tmpdir}"

XDT = mybir.dt.float16
F32 = mybir.dt.float32
NP_XDT = np.float16

C = 96          # channels (in == out)
B = 8           # num basis
K = 3           # kernel size
N_CORES = 8
H = W = 512     # input spatial
HO = WO = 256   # output spatial
HP = H + 2      # padded rows
WP = W + 2      # padded cols (phase-split: [257 even | 257 odd])
SH_ROWS = 257   # padded rows per shard (256 + 1 halo)
CORE_HO = 128   # output rows per core
BH = 32         # output rows per block
NBLK = CORE_HO // BH
IN_ROWS = 2 * BH + 1  # input rows per block (65)
# input-chunk row splits within a block (finer at the front so the PE can
# start as soon as the first rows land)
IN_CHUNKS = (17, 16, 16, 16)
# output stored as fp16 to halve write traffic (accumulation stays fp32)
OUT_BF16 = os.environ.get("KERNEL_OUT_F32", "") != "1"
# dummy matmuls issued before the first real tile to warm the PE clock gate
N_WARMUP = int(os.environ.get("KERNEL_WARMUP", "0"))

# column base per kw tap: even-phase col 2*ow -> slot ow (base 0);
# odd-phase col 2*ow+1 -> slot 257+ow; even col 2*ow+2 -> slot ow+1.
_KW_BASE = {0: 0, 1: 257, 2: 1}

_PROGRAM_CACHE = {}


def _build_program():
    """One SPMD Bass program, shared by all 8 cores."""
    nc = bacc.Bacc()
    out_dt = XDT if OUT_BF16 else F32
    x_d = nc.dram_tensor("x", [C, SH_ROWS, WP], XDT, kind="ExternalInput")
    w_d = nc.dram_tensor("wt", [C, K * K, C], XDT, kind="ExternalInput")
    b_d = nc.dram_tensor("bias", [C, 1], F32, kind="ExternalInput")
    y_d = nc.dram_tensor("out", [C, CORE_HO, WO], out_dt, kind="ExternalOutput")

    with tile.TileContext(nc) as tc:
        with (
            tc.tile_pool(name="const", bufs=1) as cpool,
            tc.tile_pool(name="xin", bufs=2) as xpool,
            tc.tile_pool(name="oout", bufs=2) as opool,
            tc.tile_pool(name="ext", bufs=4) as epool,
            tc.tile_pool(name="psum", bufs=8, space=bass.MemorySpace.PSUM) as ppool,
        ):
            # constants go first on the SP HWDGE ring: tiny (166KB), they land
            # before the first x chunk with far less latency jitter than the
            # SWDGE (gpsimd) path, whose Q7 emission + completion adds 1-3us
            wt = cpool.tile([C, K * K, C], XDT)
            nc.sync.dma_start(wt[:], w_d[:])
            # bias padded to 128 partitions: partitions 96-127 get 0 so one
            # ACT pass can evacuate a full [128, 512] PSUM bank (main + spill)
            bias = cpool.tile([128, 1], F32)
            nc.gpsimd.memset(bias[:], 0)
            nc.sync.dma_start(bias[:C, :], b_d[:])

            # --- column-tiled schedule ------------------------------------
            # The PE runs in 128x32 col-tiled mode: 4 independent 128x32
            # strips, each with its own stationary weights and rhs stream.
            # M=96 out-channels only need 3 strips, so strip 3 (PSUM
            # partitions 96-127) is free capacity: per dual-row tile, one
            # rotating out-channel block ob=tau%3 sends 7 of its 9 taps to
            # strip 3, cutting rounds per tile from 9 to ~7 (4/3 speedup of
            # the matmul stream). Strip-3 partials are evacuated with the
            # same ACT pass (bias 0), shifted down to partitions 32*ob via
            # an SBUF->SBUF DMA, and merged by the (idle) DVE before flush.
            for blk in range(NBLK):
                xt = xpool.tile([C, IN_ROWS, WP], XDT)
                r0 = 2 * BH * blk
                rr = 0
                chunks = (5, 8, 8, 8, 8, 8, 10, 10) if blk == 0 else IN_CHUNKS
                for nrows in chunks:
                    nc.sync.dma_start(
                        xt[:, rr : rr + nrows, :],
                        x_d[:, r0 + rr : r0 + rr + nrows, :],
                    )
                    rr += nrows
                assert rr == IN_ROWS
                out_sb = opool.tile([128, BH, WO], out_dt)
                last = blk == NBLK - 1
                ndual = BH // 2 - 1 if last else BH // 2

                # build per-strip unit queues: unit = (tau, ob, tap)
                queues = [[] for _ in range(4)]
                for tau in range(ndual):
                    rsp = tau % 3
                    for ob in range(3):
                        if ob == rsp:
                            for tap in (0, 1):
                                queues[ob].append((tau, ob, tap))
                            for tap in range(2, K * K):
                                queues[3].append((tau, ob, tap))
                        else:
                            for tap in range(K * K):
                                queues[ob].append((tau, ob, tap))
                total_units = {}
                for s in range(4):
                    for tau, ob, tap in queues[s]:
                        total_units[(tau, s)] = total_units.get((tau, s), 0) + 1

                P = {}
                emitted = {}
                tau_left = {tau: 3 * K * K for tau in range(ndual)}

                def emit_post(tau):
                    """ACT-evacuate, shift strip-3 spill, DVE-merge, flush."""
                    rsp = tau % 3
                    nc.scalar.activation(
                        out_sb[:, 2 * tau : 2 * tau + 2, :],
                        P[tau][:].rearrange("p (a b) -> p a b", a=2),
                        mybir.ActivationFunctionType.Identity,
                        bias=bias[:],
                    )
                    # move spill rows from partitions 96-128 to 32*ob..+32
                    e2 = epool.tile([128, 2, WO], out_dt)
                    nc.sync.dma_start(
                        e2[32 * rsp : 32 * rsp + 32, :, :],
                        out_sb[96:128, 2 * tau : 2 * tau + 2, :],
                    )
                    nc.vector.scalar_tensor_tensor(
                        out_sb[32 * rsp : 32 * rsp + 32, 2 * tau : 2 * tau + 2, :],
                        e2[32 * rsp : 32 * rsp + 32, :, :],
                        1.0,
                        out_sb[32 * rsp : 32 * rsp + 32, 2 * tau : 2 * tau + 2, :],
                        op0=mybir.AluOpType.mult,
                        op1=mybir.AluOpType.add,
                    )
                    flush_at = (
                        (7, 11, 13, 14) if last else (BH // 4 - 1, BH // 2 - 1)
                    )
                    if tau in flush_at:
                        fi = flush_at.index(tau)
                        prev = 0 if fi == 0 else (flush_at[fi - 1] + 1)
                        lo, hi = 2 * prev, 2 * tau + 2
                        eng = nc.sync if (last and fi % 2 == 0) else nc.scalar
                        eng.dma_start(
                            y_d[:, BH * blk + lo : BH * blk + hi, :],
                            out_sb[:C, lo:hi, :],
                        )

                idx = [0] * 4
                remaining = sum(len(q) for q in queues)
                while remaining:
                    for s in range(4):
                        if idx[s] >= len(queues[s]):
                            continue
                        tau, ob, tap = queues[s][idx[s]]
                        idx[s] += 1
                        remaining -= 1
                        if tau not in P:
                            P[tau] = ppool.tile([128, 2 * WO], F32, tag="ps", name=f"P{blk}_{tau}")
                        key = (tau, s)
                        cnt = emitted.get(key, 0)
                        emitted[key] = cnt + 1
                        kh, kw = tap // K, tap % K
                        cb = _KW_BASE[kw]
                        col = 32 * ob if s < 3 else 96
                        nc.tensor.matmul(
                            P[tau][col : col + 32, :],
                            wt[:, tap, 32 * ob : 32 * ob + 32],
                            xt[:, 4 * tau + kh : 4 * tau + kh + 3 : 2, cb : cb + WO],
                            start=(cnt == 0),
                            stop=(cnt + 1 == total_units[key]),
                            tile_position=(0, col),
                        )
                        tau_left[tau] -= 1
                        if tau_left[tau] == 0:
                            emit_post(tau)

                if last:
                    # final two rows as plain 3-strip single-row tiles (no
                    # strip-3 spill: keeps the end-of-kernel chain short)
                    for r in (BH - 2, BH - 1):
                        psr = ppool.tile([128, WO], F32, tag="ps")
                        for ob in range(3):
                            for tap in range(K * K):
                                kh, kw = tap // K, tap % K
                                cb = _KW_BASE[kw]
                                nc.tensor.matmul(
                                    psr[32 * ob : 32 * ob + 32, :],
                                    wt[:, tap, 32 * ob : 32 * ob + 32],
                                    xt[:, 2 * r + kh, cb : cb + WO],
                                    start=(tap == 0),
                                    stop=(tap == K * K - 1),
                                    tile_position=(0, 32 * ob),
                                )
                        nc.scalar.activation(
                            out_sb[:C, r, :],
                            psr[:C, :],
                            mybir.ActivationFunctionType.Identity,
                            bias=bias[:C, :],
                        )
                        eng = nc.scalar if r == BH - 2 else nc.sync
                        eng.dma_start(
                            y_d[:, BH * blk + r, :], out_sb[:C, r, :]
                        )

    nc.compile()
    return nc


def _get_program():
    if "nc" not in _PROGRAM_CACHE:
        _PROGRAM_CACHE["nc"] = _build_program()
    return _PROGRAM_CACHE["nc"]


def _prepare_inputs(x, coeff, basis, bias):
    """Host prep: effective weights, padded phase-split fp16 x, shards."""
    x = np.asarray(x)
    coeff = np.asarray(coeff)
    basis = np.asarray(basis)
    bias = np.asarray(bias)
    # wt[i, tap, o] = sum_b coeff[o,i,b] * basis[b, tap]
    w_eff = (
        coeff.astype(np.float32).reshape(C * C, B)
        @ basis.astype(np.float32).reshape(B, K * K)
    ).reshape(C, C, K * K)
    wt = np.ascontiguousarray(w_eff.transpose(1, 2, 0)).astype(NP_XDT)

    xb = x.astype(NP_XDT)
    xph = np.zeros((x.shape[0], C, HP, WP), dtype=NP_XDT)
    # even phase: padded col 2j -> orig col 2j-1  (slot j=1..256)
    xph[:, :, 1 : H + 1, 1:257] = xb[:, :, :, 1::2]
    # odd phase: padded col 2j+1 -> orig col 2j  (slot 257+j, j=0..255)
    xph[:, :, 1 : H + 1, 257:513] = xb[:, :, :, 0::2]

    bias2 = np.ascontiguousarray(bias.astype(np.float32).reshape(C, 1))

    in_maps = []
    for s in range(N_CORES):
        b_idx, h_idx = divmod(s, 2)
        shard = np.ascontiguousarray(
            xph[b_idx, :, 256 * h_idx : 256 * h_idx + SH_ROWS, :]
        )
        in_maps.append({"x": shard, "wt": wt, "bias": bias2})
    return in_maps


def _assemble(results, n_batch):
    out = np.empty((n_batch, C, 2 * CORE_HO, WO), dtype=np.float32)
    for s in range(N_CORES):
        b_idx, h_idx = divmod(s, 2)
        out[b_idx, :, CORE_HO * h_idx : CORE_HO * (h_idx + 1), :] = results[s][
            "out"
        ].astype(np.float32)
    return out


def run(x, coeff, basis, bias, trace=False, trace_cores=None):
    """Run the kernel; returns (full_output, BassKernelResults)."""
    nc = _get_program()
    in_maps = _prepare_inputs(x, coeff, basis, bias)
    last_err = None
    for attempt in range(3):
        try:
            res = _bu.run_bass_kernel_spmd(
                nc,
                in_maps,
                list(range(N_CORES)),
                trace=trace,
                trace_cores=trace_cores,
            )
            return _assemble(res.results, x.shape[0]), res
        except Exception as e:  # transient NRT device-unrecoverable after
            last_err = e        # abrupt neighbor-process exits; nudge + retry
            if attempt == 2 or "UNAVAILABLE" not in str(e):
                raise
            import time

            import jax
            import jax.numpy as jnp

            time.sleep(15)
            try:
                a = jnp.ones((8, 8))
                (a @ a).block_until_ready()
            except Exception:
                time.sleep(15)
    raise last_err


def kernel(x, coeff, basis, bias):
    out, _ = run(x, coeff, basis, bias, trace=False)
    return out



# revision 28
# speedup vs baseline: 1.0527x; 1.0527x over previous
"""DISCO downsample conv (3x3, stride 2, pad 1) on 8 Trainium2 NeuronCores.

Strategy:
  - Effective weights w[o,i,kh,kw] = sum_b coeff[o,i,b]*basis[b,kh,kw] are tiny:
    computed on host, shipped per-tap transposed as wt[i, tap, o] (fp16).
  - x is zero-padded (H+2, W+2) on host, W phase-split into [even | odd]
    columns so every conv tap reads a contiguous run of 256 columns, and cast
    to fp16 (halves HBM traffic vs fp32 at the same 1 cycle/row TensorE rate
    as bf16, with 8x finer mantissa; PSUM accumulation stays fp32).
  - Sharding: 8 shards = (batch b in 0..3) x (H half in 0..1). Each core gets
    padded rows [256*h, 256*h + 257) of batch b -- the 1-row halo is part of
    the shard, so no inter-core communication is needed.
  - Per core: stream 4 row-blocks (32 output rows each = 65 input rows),
    each loaded in chunks so compute overlaps the load. For each pair of
    output rows: one PSUM bank [96, 512], accumulate 9 matmuls (K=96
    in-channels, M=96 out-channels, N=512 pixels), then ScalarE adds bias
    while copying PSUM -> SBUF (fp16), and rows are flushed to HBM in
    half-blocks on the ACT HWDGE ring.
"""

import os
import sys
import types

import numpy as np


# ----------------------------------------------------------------------------
# Environment bootstrap (self-contained: no reads from /root/problem).
# ----------------------------------------------------------------------------
def _ensure_paths():
    for p in (
        "/root/.axon_site",
        "/root/.axon_site/_ro/trn_rl_repo",
        "/root/.axon_site/_ro/pypackages",
        "/opt/trn_rl_repo",
    ):
        if os.path.isdir(p) and p not in sys.path:
            sys.path.append(p)


_ensure_paths()

import ml_dtypes  # noqa: E402


def _install_ntff_hook():
    """Register the NTFF profile hook (used when tracing; harmless otherwise)."""
    try:
        import antenv
    except ImportError:
        return
    if "antenv.axon_hooks" not in sys.modules:
        hooks_mod = types.ModuleType("antenv.axon_hooks")
        _hook = [None]
        hooks_mod.set_axon_ntff_profile_hook = lambda h: _hook.__setitem__(0, h)
        hooks_mod.get_axon_ntff_profile_hook = lambda: _hook[0]
        sys.modules["antenv.axon_hooks"] = hooks_mod
        antenv.axon_hooks = hooks_mod
    from antenv.axon_hooks import (
        get_axon_ntff_profile_hook,
        set_axon_ntff_profile_hook,
    )

    if get_axon_ntff_profile_hook() is None:
        try:
            from trn_agent_boot.trn_boot import _ntff_profile_via_ctypes

            so = "/opt/axon/libaxon_pjrt.so"
            if os.path.exists(so):
                set_axon_ntff_profile_hook(_ntff_profile_via_ctypes(so))
        except Exception:
            pass


_install_ntff_hook()

import concourse.bass as bass  # noqa: E402
import concourse.tile as tile  # noqa: E402
from concourse import bacc, mybir  # noqa: E402
import concourse.bass_utils as _bu  # noqa: E402

# Artifact upload needs a bucket that isn't reachable here; keep traces local.
_bu.upload_artifacts = lambda tmpdir: f"local:{tmpdir}"

XDT = mybir.dt.float16
F32 = mybir.dt.float32
NP_XDT = np.float16

C = 96          # channels (in == out)
B = 8           # num basis
K = 3           # kernel size
N_CORES = 8
H = W = 512     # input spatial
HO = WO = 256   # output spatial
HP = H + 2      # padded rows
WP = W + 2      # padded cols (phase-split: [257 even | 257 odd])
SH_ROWS = 257   # padded rows per shard (256 + 1 halo)
CORE_HO = 128   # output rows per core
BH = 32         # output rows per block
NBLK = CORE_HO // BH
IN_ROWS = 2 * BH + 1  # input rows per block (65)
# input-chunk row splits within a block (finer at the front so the PE can
# start as soon as the first rows land)
IN_CHUNKS = (17, 16, 16, 16)
# output stored as fp16 to halve write traffic (accumulation stays fp32)
OUT_BF16 = os.environ.get("KERNEL_OUT_F32", "") != "1"
# dummy matmuls issued before the first real tile to warm the PE clock gate
N_WARMUP = int(os.environ.get("KERNEL_WARMUP", "0"))

# column base per kw tap: even-phase col 2*ow -> slot ow (base 0);
# odd-phase col 2*ow+1 -> slot 257+ow; even col 2*ow+2 -> slot ow+1.
_KW_BASE = {0: 0, 1: 257, 2: 1}

_PROGRAM_CACHE = {}


def _build_program():
    """One SPMD Bass program, shared by all 8 cores."""
    nc = bacc.Bacc()
    out_dt = XDT if OUT_BF16 else F32
    x_d = nc.dram_tensor("x", [C, SH_ROWS, WP], XDT, kind="ExternalInput")
    w_d = nc.dram_tensor("wt", [C, K * K, C], XDT, kind="ExternalInput")
    b_d = nc.dram_tensor("bias", [C, 1], F32, kind="ExternalInput")
    y_d = nc.dram_tensor("out", [C, CORE_HO, WO], out_dt, kind="ExternalOutput")

    with tile.TileContext(nc) as tc:
        with (
            tc.tile_pool(name="const", bufs=1) as cpool,
            tc.tile_pool(name="xin", bufs=2) as xpool,
            tc.tile_pool(name="oout", bufs=2) as opool,
            tc.tile_pool(name="ext", bufs=2) as epool,
            tc.tile_pool(name="psum", bufs=8, space=bass.MemorySpace.PSUM) as ppool,
        ):
            # constants go first on the SP HWDGE ring: tiny (166KB), they land
            # before the first x chunk with far less latency jitter than the
            # SWDGE (gpsimd) path, whose Q7 emission + completion adds 1-3us
            wt = cpool.tile([C, K * K, C], XDT)
            nc.sync.dma_start(wt[:], w_d[:])
            # bias padded to 128 partitions: partitions 96-127 get 0 so one
            # ACT pass can evacuate a full [128, 512] PSUM bank (main + spill)
            bias = cpool.tile([128, 1], F32)
            nc.gpsimd.memset(bias[:], 0)
            nc.sync.dma_start(bias[:C, :], b_d[:])

            # --- column-tiled schedule ------------------------------------
            # The PE runs in 128x32 col-tiled mode: 4 independent 128x32
            # strips, each with its own stationary weights and rhs stream.
            # M=96 out-channels only need 3 strips, so strip 3 (PSUM
            # partitions 96-127) is free capacity: per dual-row tile, one
            # rotating out-channel block ob=tau%3 sends 7 of its 9 taps to
            # strip 3, cutting rounds per tile from 9 to ~7 (4/3 speedup of
            # the matmul stream). Strip-3 partials are evacuated with the
            # same ACT pass (bias 0), shifted down to partitions 32*ob via
            # an SBUF->SBUF DMA, and merged by the (idle) DVE before flush.
            for blk in range(NBLK):
                xt = xpool.tile([C, IN_ROWS, WP], XDT)
                r0 = 2 * BH * blk
                rr = 0
                chunks = (5, 8, 8, 8, 8, 8, 10, 10) if blk == 0 else IN_CHUNKS
                for nrows in chunks:
                    nc.sync.dma_start(
                        xt[:, rr : rr + nrows, :],
                        x_d[:, r0 + rr : r0 + rr + nrows, :],
                    )
                    rr += nrows
                assert rr == IN_ROWS
                out_sb = opool.tile([128, BH, WO], out_dt)
                last = blk == NBLK - 1
                ndual = BH // 2 - 1 if last else BH // 2

                # flush-groups of ~5 dual-row tiles; every tile in group g
                # spills the SAME out-channel block rsp=g%3 to strip 3, so the
                # group's spill rows move down in ONE batched DMA + one DVE add
                groups = [(0, 5), (5, 10), (10, ndual)]
                gof = {}
                for gi, (glo, ghi) in enumerate(groups):
                    for tau in range(glo, ghi):
                        gof[tau] = gi

                # build per-strip unit queues: unit = (tau, ob, tap)
                queues = [[] for _ in range(4)]
                for tau in range(ndual):
                    rsp = gof[tau] % 3
                    for ob in range(3):
                        if ob == rsp:
                            for tap in (0, 1):
                                queues[ob].append((tau, ob, tap))
                            for tap in range(2, K * K):
                                queues[3].append((tau, ob, tap))
                        else:
                            for tap in range(K * K):
                                queues[ob].append((tau, ob, tap))
                total_units = {}
                for s in range(4):
                    for tau, ob, tap in queues[s]:
                        total_units[(tau, s)] = total_units.get((tau, s), 0) + 1

                P = {}
                emitted = {}
                tau_left = {tau: 3 * K * K for tau in range(ndual)}
                acted = set()

                def emit_group_post(gi):
                    """Batched spill shift + DVE merge + flush for group gi."""
                    glo, ghi = groups[gi]
                    rsp = gi % 3
                    lo, hi = 2 * glo, 2 * ghi
                    nrow = hi - lo
                    p0 = 32 * rsp
                    e2 = epool.tile(
                        [128, 12, WO], out_dt, tag="e2", name=f"e2_{blk}_{gi}"
                    )
                    # spill rows down from partitions 96-128 to the ob block
                    # on the idle SWDGE ring (keeps both HWDGE rings clear)
                    nc.gpsimd.dma_start(
                        e2[p0 : p0 + 32, :nrow, :],
                        out_sb[96:128, lo:hi, :],
                    )
                    nc.vector.scalar_tensor_tensor(
                        out_sb[p0 : p0 + 32, lo:hi, :],
                        e2[p0 : p0 + 32, :nrow, :],
                        1.0,
                        out_sb[p0 : p0 + 32, lo:hi, :],
                        op0=mybir.AluOpType.mult,
                        op1=mybir.AluOpType.add,
                    )
                    eng = nc.sync if (last and gi % 2 == 0) else nc.scalar
                    eng.dma_start(
                        y_d[:, BH * blk + lo : BH * blk + hi, :],
                        out_sb[:C, lo:hi, :],
                    )

                idx = [0] * 4
                remaining = sum(len(q) for q in queues)
                while remaining:
                    for s in range(4):
                        if idx[s] >= len(queues[s]):
                            continue
                        tau, ob, tap = queues[s][idx[s]]
                        idx[s] += 1
                        remaining -= 1
                        if tau not in P:
                            P[tau] = ppool.tile(
                                [128, 2 * WO], F32, tag="ps", name=f"P{blk}_{tau}"
                            )
                        key = (tau, s)
                        cnt = emitted.get(key, 0)
                        emitted[key] = cnt + 1
                        kh, kw = tap // K, tap % K
                        cb = _KW_BASE[kw]
                        col = 32 * ob if s < 3 else 96
                        nc.tensor.matmul(
                            P[tau][col : col + 32, :],
                            wt[:, tap, 32 * ob : 32 * ob + 32],
                            xt[:, 4 * tau + kh : 4 * tau + kh + 3 : 2, cb : cb + WO],
                            start=(cnt == 0),
                            stop=(cnt + 1 == total_units[key]),
                            tile_position=(0, col),
                        )
                        tau_left[tau] -= 1
                        if tau_left[tau] == 0:
                            # evacuate the full bank (bias rows 96-127 are 0)
                            nc.scalar.activation(
                                out_sb[:, 2 * tau : 2 * tau + 2, :],
                                P[tau][:].rearrange("p (a b) -> p a b", a=2),
                                mybir.ActivationFunctionType.Identity,
                                bias=bias[:],
                            )
                            acted.add(tau)
                            gi = gof[tau]
                            glo, ghi = groups[gi]
                            if all(t in acted for t in range(glo, ghi)):
                                emit_group_post(gi)

                if last:
                    # final two rows as plain 3-strip single-row tiles (no
                    # strip-3 spill: keeps the end-of-kernel chain short)
                    for r in (BH - 2, BH - 1):
                        psr = ppool.tile([128, WO], F32, tag="ps")
                        for ob in range(3):
                            for tap in range(K * K):
                                kh, kw = tap // K, tap % K
                                cb = _KW_BASE[kw]
                                nc.tensor.matmul(
                                    psr[32 * ob : 32 * ob + 32, :],
                                    wt[:, tap, 32 * ob : 32 * ob + 32],
                                    xt[:, 2 * r + kh, cb : cb + WO],
                                    start=(tap == 0),
                                    stop=(tap == K * K - 1),
                                    tile_position=(0, 32 * ob),
                                )
                        nc.scalar.activation(
                            out_sb[:C, r, :],
                            psr[:C, :],
                            mybir.ActivationFunctionType.Identity,
                            bias=bias[:C, :],
                        )
                        eng = nc.scalar if r == BH - 2 else nc.sync
                        eng.dma_start(
                            y_d[:, BH * blk + r, :], out_sb[:C, r, :]
                        )

    nc.compile()
    return nc


def _get_program():
    if "nc" not in _PROGRAM_CACHE:
        _PROGRAM_CACHE["nc"] = _build_program()
    return _PROGRAM_CACHE["nc"]


def _prepare_inputs(x, coeff, basis, bias):
    """Host prep: effective weights, padded phase-split fp16 x, shards."""
    x = np.asarray(x)
    coeff = np.asarray(coeff)
    basis = np.asarray(basis)
    bias = np.asarray(bias)
    # wt[i, tap, o] = sum_b coeff[o,i,b] * basis[b, tap]
    w_eff = (
        coeff.astype(np.float32).reshape(C * C, B)
        @ basis.astype(np.float32).reshape(B, K * K)
    ).reshape(C, C, K * K)
    wt = np.ascontiguousarray(w_eff.transpose(1, 2, 0)).astype(NP_XDT)

    xb = x.astype(NP_XDT)
    xph = np.zeros((x.shape[0], C, HP, WP), dtype=NP_XDT)
    # even phase: padded col 2j -> orig col 2j-1  (slot j=1..256)
    xph[:, :, 1 : H + 1, 1:257] = xb[:, :, :, 1::2]
    # odd phase: padded col 2j+1 -> orig col 2j  (slot 257+j, j=0..255)
    xph[:, :, 1 : H + 1, 257:513] = xb[:, :, :, 0::2]

    bias2 = np.ascontiguousarray(bias.astype(np.float32).reshape(C, 1))

    in_maps = []
    for s in range(N_CORES):
        b_idx, h_idx = divmod(s, 2)
        shard = np.ascontiguousarray(
            xph[b_idx, :, 256 * h_idx : 256 * h_idx + SH_ROWS, :]
        )
        in_maps.append({"x": shard, "wt": wt, "bias": bias2})
    return in_maps


def _assemble(results, n_batch):
    out = np.empty((n_batch, C, 2 * CORE_HO, WO), dtype=np.float32)
    for s in range(N_CORES):
        b_idx, h_idx = divmod(s, 2)
        out[b_idx, :, CORE_HO * h_idx : CORE_HO * (h_idx + 1), :] = results[s][
            "out"
        ].astype(np.float32)
    return out


def run(x, coeff, basis, bias, trace=False, trace_cores=None):
    """Run the kernel; returns (full_output, BassKernelResults)."""
    nc = _get_program()
    in_maps = _prepare_inputs(x, coeff, basis, bias)
    last_err = None
    for attempt in range(3):
        try:
            res = _bu.run_bass_kernel_spmd(
                nc,
                in_maps,
                list(range(N_CORES)),
                trace=trace,
                trace_cores=trace_cores,
            )
            return _assemble(res.results, x.shape[0]), res
        except Exception as e:  # transient NRT device-unrecoverable after
            last_err = e        # abrupt neighbor-process exits; nudge + retry
            if attempt == 2 or "UNAVAILABLE" not in str(e):
                raise
            import time

            import jax
            import jax.numpy as jnp

            time.sleep(15)
            try:
                a = jnp.ones((8, 8))
                (a @ a).block_until_ready()
            except Exception:
                time.sleep(15)
    raise last_err


def kernel(x, coeff, basis, bias):
    out, _ = run(x, coeff, basis, bias, trace=False)
    return out

